# revision 5
# baseline (speedup 1.0000x reference)
"""Masked-loss kernel for nn_MLoss_9715216024200 on 8 Trainium2 NeuronCores.

loss = sum(where(y[...,0]>0.5, (y-x)^2 - a*x^2, 0)) + a*sum(x[...,0]^2)
with x,y f32 (256, 10647, 5); output is a f32 scalar.

Sharding: flatten to cells (5 contiguous values each), pad with 256 zero
cells (neutral: y0=0 -> mask 0, x=0 -> no bg term), reshape to
(8 cores, 128 partitions, 2662 cells), and ship the shards as bf16 --
the loss tolerates bf16 inputs (rel err ~1e-4 << 2e-2) and it halves the
HBM stream to 13310 B/partition/tensor (~19us at the 360 GB/s DMA
roofline, which this kernel saturates).

Per-core math uses mask idempotence (m in {0,1} => m^2 = m):

  sum(m*(d^2 - a*x^2)) = sum((m*y)^2) - 2*sum((m*x) o y) + (1-a)*sum((m*x)^2)

so only TWO masked tensors (my = m*y, mx = m*x) are ever materialized --
in a SINGLE DVE tensor_tensor per tile: x and y live in one SBUF tile
xy=[x|y], and the mask multiplies both halves through an outer-broadcast
AP ([P,2,n] view with the mask's outer stride 0), which keeps the packed
bf16 2x DVE rate.  Work spreads across ALL FIVE engines, each under the
~19us DMA stream:

  Pool+DVE: m5 = bf16(y0 > 0.5) replicated to 5 features (Pool 1.44/elem
            takes the early/middle tiles, DVE 0.52/elem the late ones)
  DVE:      mxy = xy * m5b  (one 2n-elem bf16 tensor_tensor, 0.52/elem)
  ACT:      sum((my)^2) via fused Square+accum  (0.83/elem + ~0.6us/instr)
  PE:       sum(mx o y), sum((mx)^2), sum(x0^2) as Gram diagonals: per
            128-col block, matmul(psA+=mx^T y), matmul(psB+=mx^T x), and
            per 128-cell block matmul(psC+=x0^T x0), all accumulating in
            one PSUM tile (53ns/matmul; the trace of each 128x128 Gram is
            taken on the host from a single staged [P,384] export).

The last LAST_VEC tiles skip PE (cross term via DVE ttr, squares on ACT)
so the Grams close early and the PSUM export overlaps the tail.
Host combines in f64.
"""
import sys

for _p in ('/opt/trn_rl_repo',):
    if _p in sys.path:
        sys.path.remove(_p)
    sys.path.insert(0, _p)

import os as _os
import numpy as np

B, C, F = 256, 10647, 5
THRESH = 0.5
ALPHA = 0.1
N_CORES = 8
P = 128
CELLS = B * C                      # 2,725,632
CELLS_PER_PART = 2662              # 8*128*2662 = 2,725,888
PAD_CELLS = N_CORES * P * CELLS_PER_PART - CELLS   # 256
FD = CELLS_PER_PART * F            # 13310 elems per partition per core

_ts = _os.environ.get('TILE_SIZES', '')
TILE_SIZES = ([int(v) for v in _ts.split(',')] if _ts
              else [128, 256, 512, 512, 512, 448, 192, 102])
assert sum(TILE_SIZES) == CELLS_PER_PART
N_TILES = len(TILE_SIZES)
# per-tile fraction of the mask computed on Pool (rest on DVE)
_pf = _os.environ.get('POOL_FRAC', '')
POOL_FRAC = ([float(v) for v in _pf.split(',')] if _pf
             else [1.0, 0.6, 0.6, 0.6, 0.6, 0.6, 0.6, 1.0])
assert len(POOL_FRAC) == N_TILES
# tiles whose sum((my)^2) runs on PE instead of ACT
_mp = _os.environ.get('MYSQ_PE', '')
MYSQ_PE = set(int(v) for v in _mp.split(',') if v != '')
# how many trailing tiles skip PE (cross term on DVE ttr, squares on ACT)
LAST_VEC = int(_os.environ.get('LAST_VEC', '1'))
# PE emission order head: backlog these tiles first so PE ramps once
_pe0 = _os.environ.get('PE_HEAD', '2')
PE_HEAD = [int(v) for v in _pe0.split(',') if v != '']
BUFS = [int(v) for v in _os.environ.get('BUFS', '6,4,4,2').split(',')]

_compiled = None


def _build():
    from contextlib import ExitStack
    import concourse.tile as tile
    from concourse import bacc, mybir

    sqa = float(np.sqrt(ALPHA))

    nc = bacc.Bacc("TRN2", target_bir_lowering=False, debug=False,
                   enable_asserts=True, num_devices=N_CORES)
    bf16 = mybir.dt.bfloat16
    f32 = mybir.dt.float32
    x_d = nc.dram_tensor("x", [P, FD], bf16, kind="ExternalInput").ap()
    y_d = nc.dram_tensor("y", [P, FD], bf16, kind="ExternalInput").ap()
    o_d = nc.dram_tensor("o", [P, 4 * N_TILES], f32, kind="ExternalOutput").ap()
    g_d = nc.dram_tensor("g", [P, 512], f32, kind="ExternalOutput").ap()

    Sq = mybir.ActivationFunctionType.Square
    Alu = mybir.AluOpType

    # psum column ranges: A=cross(mx,y)  B=(mx)^2  C=bg x0^2  D=(my)^2-on-PE
    first_pe = [True, True, True, True]
    n_pe_tiles = N_TILES - LAST_VEC

    with tile.TileContext(nc) as tc, ExitStack() as ctx:
        xyp = ctx.enter_context(tc.tile_pool(name="xy", bufs=BUFS[0]))
        mp_ = ctx.enter_context(tc.tile_pool(name="m", bufs=BUFS[1]))
        wp = ctx.enter_context(tc.tile_pool(name="w", bufs=BUFS[2]))
        sp = ctx.enter_context(tc.tile_pool(name="s", bufs=BUFS[3]))
        ap_ = ctx.enter_context(tc.tile_pool(name="acc", bufs=1))
        pp = ctx.enter_context(tc.psum_pool(name="ps", bufs=1))

        acc = ap_.tile([P, 4 * N_TILES], f32)
        gst = ap_.tile([P, 512], f32)
        ps = pp.tile([P, 512], f32)

        def emit_pe(t, cells, fd, xv, yv, mxv, myv):
            last_pe_tile = (t == n_pe_tiles - 1)
            nb = (fd + 127) // 128
            for j in range(nb):
                lo = j * 128
                w = min(128, fd - lo)
                is_last = last_pe_tile and (j == nb - 1)
                nc.tensor.matmul(ps[0:w, 0:w], mxv[:, lo:lo + w],
                                 yv[:, lo:lo + w],
                                 start=first_pe[0], stop=is_last,
                                 skip_group_check=True)
                first_pe[0] = False
                nc.tensor.matmul(ps[0:w, 128:128 + w], mxv[:, lo:lo + w],
                                 xv[:, lo:lo + w],
                                 start=first_pe[1], stop=is_last,
                                 skip_group_check=True)
                first_pe[1] = False
                if t in MYSQ_PE:
                    nc.tensor.matmul(ps[0:w, 384:384 + w],
                                     myv[:, lo:lo + w], myv[:, lo:lo + w],
                                     start=first_pe[3], stop=is_last,
                                     skip_group_check=True)
                    first_pe[3] = False
            # background: x0 (stride-5 view) Gram over 128-cell blocks
            x0v = xv[:, 0::F]
            nbc = (cells + 127) // 128
            for j in range(nbc):
                lo = j * 128
                w = min(128, cells - lo)
                is_last = last_pe_tile and (j == nbc - 1)
                nc.tensor.matmul(ps[0:w, 256:256 + w], x0v[:, lo:lo + w],
                                 x0v[:, lo:lo + w],
                                 start=first_pe[2], stop=is_last,
                                 skip_group_check=True)
                first_pe[2] = False
            if last_pe_tile:
                # stage Grams to SBUF + export (overlaps the tail tiles)
                nc.vector.tensor_copy(gst[:], ps[:])
                nc.scalar.dma_start(g_d, gst[:])

        # PE emission order: PE_HEAD tiles first (backlog so PE ramps once)
        pe_order = PE_HEAD + [t for t in range(n_pe_tiles) if t not in PE_HEAD]
        pe_pending = {}     # tile -> emit args, not yet flushed
        pe_next = 0         # index into pe_order of next closure to emit

        off = 0
        for t, cells in enumerate(TILE_SIZES):
            fd = cells * F
            xy = xyp.tile([P, 2 * fd], bf16, tag="xy")
            xv = xy[:, 0:fd]
            yv = xy[:, fd:2 * fd]
            sl = slice(off, off + fd)
            off += fd
            nc.sync.dma_start(yv, y_d[:, sl])
            nc.sync.dma_start(xv, x_d[:, sl])

            # mask replicated to all 5 features, split Pool/DVE by POOL_FRAC
            m5 = mp_.tile([P, fd], bf16, tag="m5")
            pc = int(round(cells * POOL_FRAC[t]))
            for (clo, chi, eng) in ((0, pc, nc.gpsimd), (pc, cells, nc.vector)):
                if chi <= clo:
                    continue
                w = chi - clo
                yp_ = yv[:, F * clo:F * chi]
                y0b = yp_[:, 0::F].unsqueeze(2).broadcast_to((P, w, F))
                eng.tensor_scalar(
                    m5[:, F * clo:F * chi].rearrange("p (k f) -> p k f", f=F),
                    y0b, THRESH, None, op0=Alu.is_gt)

            # mxy = [mx | my] in one bf16 2x tensor_tensor
            mxy = wp.tile([P, 2 * fd], bf16, tag="mxy")
            m5b = m5[:].unsqueeze(1).broadcast_to((P, 2, fd))
            nc.vector.tensor_tensor(
                mxy[:].rearrange("p (k n) -> p k n", k=2),
                xy[:].rearrange("p (k n) -> p k n", k=2),
                m5b, op=Alu.mult)
            mxv = mxy[:, 0:fd]
            myv = mxy[:, fd:2 * fd]

            if t not in MYSQ_PE or t >= n_pe_tiles:
                sq = sp.tile([P, fd], bf16, tag="sq")
                nc.scalar.activation(sq[:], myv, Sq, accum_out=acc[:, t:t + 1])

            if t < n_pe_tiles:
                pe_pending[t] = (t, cells, fd, xv, yv, mxv, myv)
                while pe_next < len(pe_order) and pe_order[pe_next] in pe_pending:
                    emit_pe(*pe_pending.pop(pe_order[pe_next]))
                    pe_next += 1
            else:
                # tail tile off PE: cross on DVE ttr, squares + bg on ACT
                cw = sp.tile([P, fd], bf16, tag="cw")
                nc.vector.tensor_tensor_reduce(
                    cw[:], mxv, yv, 1.0, 0.0,
                    op0=Alu.mult, op1=Alu.add,
                    accum_out=acc[:, N_TILES + t:N_TILES + t + 1])
                sq2 = sp.tile([P, fd], bf16, tag="sq2")
                nc.scalar.activation(sq2[:], mxv, Sq,
                                     accum_out=acc[:, 2 * N_TILES + t:
                                                   2 * N_TILES + t + 1])
                sq3 = sp.tile([P, cells], bf16, tag="sq3")
                nc.scalar.activation(sq3[:], xv[:, 0::F], Sq, scale=sqa,
                                     accum_out=acc[:, 3 * N_TILES + t:
                                                   3 * N_TILES + t + 1])

        nc.scalar.dma_start(o_d, acc[:])

    nc.compile()
    return nc


def _shard(a: np.ndarray) -> list[np.ndarray]:
    import ml_dtypes
    flat = a.reshape(-1)
    pad = np.zeros(PAD_CELLS * F, dtype=a.dtype)
    flat = np.concatenate([flat, pad]).astype(ml_dtypes.bfloat16)
    per_core = flat.reshape(N_CORES, P, FD)
    return [np.ascontiguousarray(per_core[i]) for i in range(N_CORES)]


def kernel(x: np.ndarray, y: np.ndarray) -> np.ndarray:
    global _compiled
    if _compiled is None:
        _compiled = _build()
    nc = _compiled

    from concourse.bass_utils import run_bass_kernel_spmd

    xs = _shard(np.asarray(x, dtype=np.float32))
    ys = _shard(np.asarray(y, dtype=np.float32))
    in_maps = [{"x": xs[i], "y": ys[i]} for i in range(N_CORES)]
    res = run_bass_kernel_spmd(nc, in_maps, core_ids=list(range(N_CORES)))

    T = N_TILES
    n_pe_tiles = T - LAST_VEC
    total = np.float64(0.0)
    for r in res.results:
        o = r["o"].astype(np.float64)
        g = r["g"].astype(np.float64)
        trA = np.trace(g[:, 0:128])
        trB = np.trace(g[:, 128:256])
        trC = np.trace(g[:, 256:384])
        trD = np.trace(g[:, 384:512])
        # sum (my)^2: ACT cols for non-PE tiles, psD for MYSQ_PE tiles
        myq = sum(o[:, t].sum() for t in range(T)
                  if t not in MYSQ_PE or t >= n_pe_tiles) + trD
        cross = trA + sum(o[:, T + t].sum() for t in range(n_pe_tiles, T))
        mxq = trB + sum(o[:, 2 * T + t].sum() for t in range(n_pe_tiles, T))
        bg = ALPHA * trC + sum(o[:, 3 * T + t].sum()
                               for t in range(n_pe_tiles, T))
        total += myq - 2.0 * cross + (1.0 - ALPHA) * mxq + bg
    return np.float32(total)


# revision 9
# speedup vs baseline: 1.1031x; 1.1031x over previous
"""Masked-loss kernel for nn_MLoss_9715216024200 on 8 Trainium2 NeuronCores.

loss = sum(where(y[...,0]>0.5, (y-x)^2 - a*x^2, 0)) + a*sum(x[...,0]^2)
with x,y f32 (256, 10647, 5); output is a f32 scalar.

Sharding: flatten to cells (5 contiguous values each), pad with 256 zero
cells (neutral: y0=0 -> mask 0, x=0 -> no bg term), reshape to
(8 cores, 128 partitions, 2662 cells), and ship the shards as bf16 --
the loss tolerates bf16 inputs (rel err ~1e-4 << 2e-2) and it halves the
HBM stream to 13310 B/partition/tensor (~19us at the 360 GB/s DMA
roofline, which this kernel saturates).

Per-core math uses mask idempotence (m in {0,1} => m^2 = m):

  sum(m*(d^2 - a*x^2)) = sum((m*y)^2) - 2*sum((m*x) o y) + (1-a)*sum((m*x)^2)

so only TWO masked tensors (my = m*y, mx = m*x) are ever materialized --
in a SINGLE DVE tensor_tensor per tile: x and y live in one SBUF tile
xy=[x|y], and the mask multiplies both halves through an outer-broadcast
AP ([P,2,n] view with the mask's outer stride 0), which keeps the packed
bf16 2x DVE rate.  Work spreads across ALL FIVE engines, each under the
~19us DMA stream:

  Pool+DVE: m5 = bf16(y0 > 0.5) replicated to 5 features (Pool 1.44/elem
            takes the early/middle tiles, DVE 0.52/elem the late ones)
  DVE:      mxy = xy * m5b  (one 2n-elem bf16 tensor_tensor, 0.52/elem)
  ACT:      sum((my)^2) via fused Square+accum  (0.83/elem + ~0.6us/instr)
  PE:       sum(mx o y), sum((mx)^2), sum(x0^2) as Gram diagonals: per
            128-col block, matmul(psA+=mx^T y), matmul(psB+=mx^T x), and
            per 128-cell block matmul(psC+=x0^T x0), all accumulating in
            one PSUM tile (53ns/matmul; the trace of each 128x128 Gram is
            taken on the host from a single staged [P,384] export).

The last LAST_VEC tiles skip PE (cross term via DVE ttr, squares on ACT)
so the Grams close early and the PSUM export overlaps the tail.
Host combines in f64.
"""
import sys

for _p in ('/opt/trn_rl_repo',):
    if _p in sys.path:
        sys.path.remove(_p)
    sys.path.insert(0, _p)

import os as _os
import numpy as np

B, C, F = 256, 10647, 5
THRESH = 0.5
ALPHA = 0.1
N_CORES = 8
P = 128
CELLS = B * C                      # 2,725,632
CELLS_PER_PART = 2662              # 8*128*2662 = 2,725,888
PAD_CELLS = N_CORES * P * CELLS_PER_PART - CELLS   # 256
FD = CELLS_PER_PART * F            # 13310 elems per partition per core

_ts = _os.environ.get('TILE_SIZES', '')
TILE_SIZES = ([int(v) for v in _ts.split(',')] if _ts
              else [128] + [256] * 9 + [128, 102])
assert sum(TILE_SIZES) == CELLS_PER_PART
N_TILES = len(TILE_SIZES)
# per-tile fraction of the mask computed on Pool (rest on DVE);
# a single value is broadcast to all tiles
_pf = _os.environ.get('POOL_FRAC', '0.75')
POOL_FRAC = [float(v) for v in _pf.split(',')]
if len(POOL_FRAC) == 1:
    POOL_FRAC = POOL_FRAC * N_TILES
assert len(POOL_FRAC) == N_TILES
# tiles whose sum((my)^2) runs on PE instead of ACT
_mp = _os.environ.get('MYSQ_PE', '1,3,5,7')
MYSQ_PE = set(int(v) for v in _mp.split(',') if v != '')
# how many trailing tiles skip PE (cross term on DVE ttr, squares on ACT)
LAST_VEC = int(_os.environ.get('LAST_VEC', '1'))
# flush tile t's PE matmuls after tile t+PE_LAG's mxy: keeps a backlog so
# PE ramps to full clock once and never idles between tiles
PE_LAG = int(_os.environ.get('PE_LAG', '2'))
BUFS = [int(v) for v in _os.environ.get('BUFS', '8,5,5,3').split(',')]

_compiled = None


def _build():
    from contextlib import ExitStack
    import concourse.tile as tile
    from concourse import bacc, mybir

    sqa = float(np.sqrt(ALPHA))

    nc = bacc.Bacc("TRN2", target_bir_lowering=False, debug=False,
                   enable_asserts=True, num_devices=N_CORES)
    bf16 = mybir.dt.bfloat16
    f32 = mybir.dt.float32
    x_d = nc.dram_tensor("x", [P, FD], bf16, kind="ExternalInput").ap()
    y_d = nc.dram_tensor("y", [P, FD], bf16, kind="ExternalInput").ap()
    o_d = nc.dram_tensor("o", [P, 4 * N_TILES], f32, kind="ExternalOutput").ap()
    g_d = nc.dram_tensor("g", [P, 512], f32, kind="ExternalOutput").ap()

    Sq = mybir.ActivationFunctionType.Square
    Alu = mybir.AluOpType

    # psum column ranges: A=cross(mx,y)  B=(mx)^2  C=bg x0^2  D=(my)^2-on-PE
    first_pe = [True, True, True, True]
    n_pe_tiles = N_TILES - LAST_VEC

    with tile.TileContext(nc) as tc, ExitStack() as ctx:
        xyp = ctx.enter_context(tc.tile_pool(name="xy", bufs=BUFS[0]))
        mp_ = ctx.enter_context(tc.tile_pool(name="m", bufs=BUFS[1]))
        wp = ctx.enter_context(tc.tile_pool(name="w", bufs=BUFS[2]))
        sp = ctx.enter_context(tc.tile_pool(name="s", bufs=BUFS[3]))
        ap_ = ctx.enter_context(tc.tile_pool(name="acc", bufs=1))
        pp = ctx.enter_context(tc.psum_pool(name="ps", bufs=1))

        acc = ap_.tile([P, 4 * N_TILES], f32)
        gst = ap_.tile([P, 512], f32)
        ps = pp.tile([P, 512], f32)

        def emit_pe(t, cells, fd, xv, yv, mxv, myv):
            last_pe_tile = (t == n_pe_tiles - 1)
            nb = (fd + 127) // 128
            for j in range(nb):
                lo = j * 128
                w = min(128, fd - lo)
                is_last = last_pe_tile and (j == nb - 1)
                nc.tensor.matmul(ps[0:w, 0:w], mxv[:, lo:lo + w],
                                 yv[:, lo:lo + w],
                                 start=first_pe[0], stop=is_last,
                                 skip_group_check=True)
                first_pe[0] = False
                nc.tensor.matmul(ps[0:w, 128:128 + w], mxv[:, lo:lo + w],
                                 xv[:, lo:lo + w],
                                 start=first_pe[1], stop=is_last,
                                 skip_group_check=True)
                first_pe[1] = False
                if t in MYSQ_PE:
                    nc.tensor.matmul(ps[0:w, 384:384 + w],
                                     myv[:, lo:lo + w], myv[:, lo:lo + w],
                                     start=first_pe[3], stop=is_last,
                                     skip_group_check=True)
                    first_pe[3] = False
            # background: x0 (stride-5 view) Gram over 128-cell blocks
            x0v = xv[:, 0::F]
            nbc = (cells + 127) // 128
            for j in range(nbc):
                lo = j * 128
                w = min(128, cells - lo)
                is_last = last_pe_tile and (j == nbc - 1)
                nc.tensor.matmul(ps[0:w, 256:256 + w], x0v[:, lo:lo + w],
                                 x0v[:, lo:lo + w],
                                 start=first_pe[2], stop=is_last,
                                 skip_group_check=True)
                first_pe[2] = False
            if last_pe_tile:
                # stage Grams to SBUF + export (overlaps the tail tiles)
                nc.vector.tensor_copy(gst[:], ps[:])
                nc.scalar.dma_start(g_d, gst[:])

        pe_pending = []     # deferred per-tile PE emissions (FIFO)

        off = 0
        for t, cells in enumerate(TILE_SIZES):
            fd = cells * F
            xy = xyp.tile([P, 2 * fd], bf16, tag="xy")
            xv = xy[:, 0:fd]
            yv = xy[:, fd:2 * fd]
            sl = slice(off, off + fd)
            off += fd
            nc.sync.dma_start(yv, y_d[:, sl])
            nc.sync.dma_start(xv, x_d[:, sl])

            # mask replicated to all 5 features, split Pool/DVE by POOL_FRAC
            m5 = mp_.tile([P, fd], bf16, tag="m5")
            pc = int(round(cells * POOL_FRAC[t]))
            for (clo, chi, eng) in ((0, pc, nc.gpsimd), (pc, cells, nc.vector)):
                if chi <= clo:
                    continue
                w = chi - clo
                yp_ = yv[:, F * clo:F * chi]
                y0b = yp_[:, 0::F].unsqueeze(2).broadcast_to((P, w, F))
                eng.tensor_scalar(
                    m5[:, F * clo:F * chi].rearrange("p (k f) -> p k f", f=F),
                    y0b, THRESH, None, op0=Alu.is_gt)

            # mxy = [mx | my] in one bf16 2x tensor_tensor
            mxy = wp.tile([P, 2 * fd], bf16, tag="mxy")
            m5b = m5[:].unsqueeze(1).broadcast_to((P, 2, fd))
            nc.vector.tensor_tensor(
                mxy[:].rearrange("p (k n) -> p k n", k=2),
                xy[:].rearrange("p (k n) -> p k n", k=2),
                m5b, op=Alu.mult)
            mxv = mxy[:, 0:fd]
            myv = mxy[:, fd:2 * fd]

            if t not in MYSQ_PE or t >= n_pe_tiles:
                sq = sp.tile([P, fd], bf16, tag="sq")
                nc.scalar.activation(sq[:], myv, Sq, accum_out=acc[:, t:t + 1])

            if t < n_pe_tiles:
                pe_pending.append((t, cells, fd, xv, yv, mxv, myv))
                while pe_pending and (pe_pending[0][0] + PE_LAG <= t
                                      or t == n_pe_tiles - 1):
                    emit_pe(*pe_pending.pop(0))
            else:
                # tail tile off PE: cross on DVE ttr, squares + bg on ACT
                cw = sp.tile([P, fd], bf16, tag="cw")
                nc.vector.tensor_tensor_reduce(
                    cw[:], mxv, yv, 1.0, 0.0,
                    op0=Alu.mult, op1=Alu.add,
                    accum_out=acc[:, N_TILES + t:N_TILES + t + 1])
                sq2 = sp.tile([P, fd], bf16, tag="sq2")
                nc.scalar.activation(sq2[:], mxv, Sq,
                                     accum_out=acc[:, 2 * N_TILES + t:
                                                   2 * N_TILES + t + 1])
                sq3 = sp.tile([P, cells], bf16, tag="sq3")
                nc.scalar.activation(sq3[:], xv[:, 0::F], Sq, scale=sqa,
                                     accum_out=acc[:, 3 * N_TILES + t:
                                                   3 * N_TILES + t + 1])

        nc.scalar.dma_start(o_d, acc[:])

    nc.compile()
    return nc


def _shard(a: np.ndarray) -> list[np.ndarray]:
    import ml_dtypes
    flat = a.reshape(-1)
    pad = np.zeros(PAD_CELLS * F, dtype=a.dtype)
    flat = np.concatenate([flat, pad]).astype(ml_dtypes.bfloat16)
    per_core = flat.reshape(N_CORES, P, FD)
    return [np.ascontiguousarray(per_core[i]) for i in range(N_CORES)]


def kernel(x: np.ndarray, y: np.ndarray) -> np.ndarray:
    global _compiled
    if _compiled is None:
        _compiled = _build()
    nc = _compiled

    from concourse.bass_utils import run_bass_kernel_spmd

    xs = _shard(np.asarray(x, dtype=np.float32))
    ys = _shard(np.asarray(y, dtype=np.float32))
    in_maps = [{"x": xs[i], "y": ys[i]} for i in range(N_CORES)]
    res = run_bass_kernel_spmd(nc, in_maps, core_ids=list(range(N_CORES)))

    T = N_TILES
    n_pe_tiles = T - LAST_VEC
    total = np.float64(0.0)
    for r in res.results:
        o = r["o"].astype(np.float64)
        g = r["g"].astype(np.float64)
        trA = np.trace(g[:, 0:128])
        trB = np.trace(g[:, 128:256])
        trC = np.trace(g[:, 256:384])
        trD = np.trace(g[:, 384:512])
        # sum (my)^2: ACT cols for non-PE tiles, psD for MYSQ_PE tiles
        myq = sum(o[:, t].sum() for t in range(T)
                  if t not in MYSQ_PE or t >= n_pe_tiles) + trD
        cross = trA + sum(o[:, T + t].sum() for t in range(n_pe_tiles, T))
        mxq = trB + sum(o[:, 2 * T + t].sum() for t in range(n_pe_tiles, T))
        bg = ALPHA * trC + sum(o[:, 3 * T + t].sum()
                               for t in range(n_pe_tiles, T))
        total += myq - 2.0 * cross + (1.0 - ALPHA) * mxq + bg
    return np.float32(total)


# revision 10
# speedup vs baseline: 1.1498x; 1.0423x over previous
"""Masked-loss kernel for nn_MLoss_9715216024200 on 8 Trainium2 NeuronCores.

loss = sum(where(y[...,0]>0.5, (y-x)^2 - a*x^2, 0)) + a*sum(x[...,0]^2)
with x,y f32 (256, 10647, 5); output is a f32 scalar.

Sharding: flatten to cells (5 contiguous values each), pad with 256 zero
cells (neutral: y0=0 -> mask 0, x=0 -> no bg term), reshape to
(8 cores, 128 partitions, 2662 cells), and ship the shards as bf16 --
the loss tolerates bf16 inputs (rel err ~1e-4 << 2e-2) and it halves the
HBM stream to 13310 B/partition/tensor (~19us at the 360 GB/s DMA
roofline, which this kernel saturates).

Per-core math uses mask idempotence (m in {0,1} => m^2 = m):

  sum(m*(d^2 - a*x^2)) = sum(m*y^2) - 2*sum((m*x) o y) + (1-a)*sum((m*x)^2)

The cross and square terms are Gram-matrix diagonals on the otherwise
idle PE: per 128-col block, matmul(psA += mx^T y), matmul(psB += mx^T x),
plus matmul(psC += x0^T x0) for the background term (53ns per matmul;
the host takes trace() of the four staged 128x128 Grams).  PE matmuls
are emitted PE_LAG tiles behind the stream so PE ramps to full clock
once and stays there.  The sum(m*y^2) term is per-tile routed to
whichever engine has slack ('act' / 'pe' / tail-'vec' mode), and the
mask m5 = bf16(y0>0.5) replicated to 5 features is split Pool/DVE by
POOL_FRAC.  Per-elem rates: Pool ts 1.45, DVE tt/ts 0.52 (packed bf16
2x), DVE ttr 1.05, ACT square 0.83 (+0.6us/instr), PE 0.41/elem/Gram.

All engines land at 10-15us, under the ~19us DMA stream; the tail tiles
(LAST_VEC) skip PE so the Gram export overlaps the drain.
Host combines in f64.
"""
import sys

for _p in ('/opt/trn_rl_repo',):
    if _p in sys.path:
        sys.path.remove(_p)
    sys.path.insert(0, _p)

import os as _os
import numpy as np

B, C, F = 256, 10647, 5
THRESH = 0.5
ALPHA = 0.1
N_CORES = 8
P = 128
CELLS = B * C                      # 2,725,632
CELLS_PER_PART = 2662              # 8*128*2662 = 2,725,888
PAD_CELLS = N_CORES * P * CELLS_PER_PART - CELLS   # 256
FD = CELLS_PER_PART * F            # 13310 elems per partition per core

_ts = _os.environ.get('TILE_SIZES', '')
TILE_SIZES = ([int(v) for v in _ts.split(',')] if _ts
              else [128] + [256] * 9 + [128, 102])
assert sum(TILE_SIZES) == CELLS_PER_PART
N_TILES = len(TILE_SIZES)
# per-tile fraction of the mask computed on Pool (rest on DVE);
# a single value is broadcast to all tiles
_pf = _os.environ.get('POOL_FRAC', '0.5')
POOL_FRAC = [float(v) for v in _pf.split(',')]
if len(POOL_FRAC) == 1:
    POOL_FRAC = POOL_FRAC * N_TILES
assert len(POOL_FRAC) == N_TILES
# per-tile engine for the sum(m*y^2) term:
#  'a' = my=y*m5 on DVE + ACT Square-accum
#  'p' = y2=y*y + PE Gram m5^T y2   (y2 on ACT if tile in Y2_ACT else DVE)
# tail LAST_VEC tiles ignore this (ttr/ACT mix, no PE)
_mm = _os.environ.get('MYSQ', 'a,p,a,p,a,p,a,p,a,p,a,a')
MYSQ = _mm.split(',')
assert len(MYSQ) == N_TILES
_ya = _os.environ.get('Y2_ACT', '1,3,5')
Y2_ACT = set(int(v) for v in _ya.split(',') if v != '')
# how many trailing tiles skip PE (cross term on DVE ttr, squares on ACT)
LAST_VEC = int(_os.environ.get('LAST_VEC', '1'))
# flush tile t's PE matmuls after tile t+PE_LAG's mxy: keeps a backlog so
# PE ramps to full clock once and never idles between tiles
PE_LAG = int(_os.environ.get('PE_LAG', '2'))
GCOPY_ACT = _os.environ.get('GCOPY_ACT', '1') == '1'
BUFS = [int(v) for v in _os.environ.get('BUFS', '8,5,5,3').split(',')]

_compiled = None


def _build():
    from contextlib import ExitStack
    import concourse.tile as tile
    from concourse import bacc, mybir

    sqa = float(np.sqrt(ALPHA))

    nc = bacc.Bacc("TRN2", target_bir_lowering=False, debug=False,
                   enable_asserts=True, num_devices=N_CORES)
    bf16 = mybir.dt.bfloat16
    f32 = mybir.dt.float32
    x_d = nc.dram_tensor("x", [P, FD], bf16, kind="ExternalInput").ap()
    y_d = nc.dram_tensor("y", [P, FD], bf16, kind="ExternalInput").ap()
    o_d = nc.dram_tensor("o", [P, 4 * N_TILES], f32, kind="ExternalOutput").ap()
    g_d = nc.dram_tensor("g", [P, 512], f32, kind="ExternalOutput").ap()

    Sq = mybir.ActivationFunctionType.Square
    Alu = mybir.AluOpType

    # psum column ranges: A=cross(mx,y)  B=(mx)^2  C=bg x0^2  D=m5 o y^2
    first_pe = [True, True, True, True]
    n_pe_tiles = N_TILES - LAST_VEC

    with tile.TileContext(nc) as tc, ExitStack() as ctx:
        xyp = ctx.enter_context(tc.tile_pool(name="xy", bufs=BUFS[0]))
        mp_ = ctx.enter_context(tc.tile_pool(name="m", bufs=BUFS[1]))
        wp = ctx.enter_context(tc.tile_pool(name="w", bufs=BUFS[2]))
        sp = ctx.enter_context(tc.tile_pool(name="s", bufs=BUFS[3]))
        ap_ = ctx.enter_context(tc.tile_pool(name="acc", bufs=1))
        pp = ctx.enter_context(tc.psum_pool(name="ps", bufs=1))

        acc = ap_.tile([P, 4 * N_TILES], f32)
        gst = ap_.tile([P, 512], f32)
        ps = pp.tile([P, 512], f32)

        def emit_pe(t, cells, fd, xv, yv, mxv, m5v, y2v):
            last_pe_tile = (t == n_pe_tiles - 1)
            nb = (fd + 127) // 128
            for j in range(nb):
                lo = j * 128
                w = min(128, fd - lo)
                is_last = last_pe_tile and (j == nb - 1)
                nc.tensor.matmul(ps[0:w, 0:w], mxv[:, lo:lo + w],
                                 yv[:, lo:lo + w],
                                 start=first_pe[0], stop=is_last,
                                 skip_group_check=True)
                first_pe[0] = False
                nc.tensor.matmul(ps[0:w, 128:128 + w], mxv[:, lo:lo + w],
                                 xv[:, lo:lo + w],
                                 start=first_pe[1], stop=is_last,
                                 skip_group_check=True)
                first_pe[1] = False
                if y2v is not None:
                    nc.tensor.matmul(ps[0:w, 384:384 + w],
                                     m5v[:, lo:lo + w], y2v[:, lo:lo + w],
                                     start=first_pe[3], stop=is_last,
                                     skip_group_check=True)
                    first_pe[3] = False
            # background: x0 (stride-5 view) Gram over 128-cell blocks
            x0v = xv[:, 0::F]
            nbc = (cells + 127) // 128
            for j in range(nbc):
                lo = j * 128
                w = min(128, cells - lo)
                is_last = last_pe_tile and (j == nbc - 1)
                nc.tensor.matmul(ps[0:w, 256:256 + w], x0v[:, lo:lo + w],
                                 x0v[:, lo:lo + w],
                                 start=first_pe[2], stop=is_last,
                                 skip_group_check=True)
                first_pe[2] = False
            if last_pe_tile:
                # stage Grams to SBUF + export (overlaps the tail tiles)
                if GCOPY_ACT:
                    nc.scalar.copy(gst[:], ps[:])
                else:
                    nc.vector.tensor_copy(gst[:], ps[:])
                nc.scalar.dma_start(g_d, gst[:])

        pe_pending = []     # deferred per-tile PE emissions (FIFO)

        off = 0
        for t, cells in enumerate(TILE_SIZES):
            fd = cells * F
            xy = xyp.tile([P, 2 * fd], bf16, tag="xy")
            xv = xy[:, 0:fd]
            yv = xy[:, fd:2 * fd]
            sl = slice(off, off + fd)
            off += fd
            nc.sync.dma_start(yv, y_d[:, sl])
            nc.sync.dma_start(xv, x_d[:, sl])

            # mask replicated to all 5 features, split Pool/DVE by POOL_FRAC
            m5 = mp_.tile([P, fd], bf16, tag="m5")
            pc = int(round(cells * POOL_FRAC[t]))
            for (clo, chi, eng) in ((0, pc, nc.gpsimd), (pc, cells, nc.vector)):
                if chi <= clo:
                    continue
                w = chi - clo
                yp_ = yv[:, F * clo:F * chi]
                y0b = yp_[:, 0::F].unsqueeze(2).broadcast_to((P, w, F))
                eng.tensor_scalar(
                    m5[:, F * clo:F * chi].rearrange("p (k f) -> p k f", f=F),
                    y0b, THRESH, None, op0=Alu.is_gt)

            mode = MYSQ[t] if t < n_pe_tiles else 'v'
            if mode == 'a' or mode == 'v':
                # mxy = [mx | my] in one bf16 2x tensor_tensor
                mxy = wp.tile([P, 2 * fd], bf16, tag="mxy")
                m5b = m5[:].unsqueeze(1).broadcast_to((P, 2, fd))
                nc.vector.tensor_tensor(
                    mxy[:].rearrange("p (k n) -> p k n", k=2),
                    xy[:].rearrange("p (k n) -> p k n", k=2),
                    m5b, op=Alu.mult)
                mxv = mxy[:, 0:fd]
                myv = mxy[:, fd:2 * fd]
                y2v = None
            else:
                # mx only; y^2 for the PE D-Gram
                mxt = wp.tile([P, fd], bf16, tag="mxy")
                nc.vector.tensor_tensor(mxt[:], xv, m5[:], op=Alu.mult)
                mxv, myv = mxt[:], None
                y2t = wp.tile([P, fd], bf16, tag="y2")
                if t in Y2_ACT:
                    nc.scalar.activation(y2t[:], yv, Sq)
                else:
                    nc.vector.tensor_tensor(y2t[:], yv, yv, op=Alu.mult)
                y2v = y2t[:]

            if mode == 'a':
                sq = sp.tile([P, fd], bf16, tag="sq")
                nc.scalar.activation(sq[:], myv, Sq, accum_out=acc[:, t:t + 1])

            if t < n_pe_tiles:
                pe_pending.append((t, cells, fd, xv, yv, mxv, m5[:], y2v))
                while pe_pending and (pe_pending[0][0] + PE_LAG <= t
                                      or t == n_pe_tiles - 1):
                    emit_pe(*pe_pending.pop(0))
            else:
                # tail tile off PE: cross + my^2 on DVE ttr, mx^2 + bg on ACT
                cw = sp.tile([P, fd], bf16, tag="cw")
                nc.vector.tensor_tensor_reduce(
                    cw[:], mxv, yv, 1.0, 0.0,
                    op0=Alu.mult, op1=Alu.add,
                    accum_out=acc[:, N_TILES + t:N_TILES + t + 1])
                cw2 = sp.tile([P, fd], bf16, tag="cw2")
                nc.vector.tensor_tensor_reduce(
                    cw2[:], myv, myv, 1.0, 0.0,
                    op0=Alu.mult, op1=Alu.add,
                    accum_out=acc[:, t:t + 1])
                sq2 = sp.tile([P, fd], bf16, tag="sq2")
                nc.scalar.activation(sq2[:], mxv, Sq,
                                     accum_out=acc[:, 2 * N_TILES + t:
                                                   2 * N_TILES + t + 1])
                sq3 = sp.tile([P, cells], bf16, tag="sq3")
                nc.scalar.activation(sq3[:], xv[:, 0::F], Sq, scale=sqa,
                                     accum_out=acc[:, 3 * N_TILES + t:
                                                   3 * N_TILES + t + 1])

        nc.scalar.dma_start(o_d, acc[:])

    nc.compile()
    return nc


def _shard(a: np.ndarray) -> list[np.ndarray]:
    import ml_dtypes
    flat = a.reshape(-1)
    pad = np.zeros(PAD_CELLS * F, dtype=a.dtype)
    flat = np.concatenate([flat, pad]).astype(ml_dtypes.bfloat16)
    per_core = flat.reshape(N_CORES, P, FD)
    return [np.ascontiguousarray(per_core[i]) for i in range(N_CORES)]


def kernel(x: np.ndarray, y: np.ndarray) -> np.ndarray:
    global _compiled
    if _compiled is None:
        _compiled = _build()
    nc = _compiled

    from concourse.bass_utils import run_bass_kernel_spmd

    xs = _shard(np.asarray(x, dtype=np.float32))
    ys = _shard(np.asarray(y, dtype=np.float32))
    in_maps = [{"x": xs[i], "y": ys[i]} for i in range(N_CORES)]
    res = run_bass_kernel_spmd(nc, in_maps, core_ids=list(range(N_CORES)))

    T = N_TILES
    n_pe_tiles = T - LAST_VEC
    total = np.float64(0.0)
    for r in res.results:
        o = r["o"].astype(np.float64)
        g = r["g"].astype(np.float64)
        trA = np.trace(g[:, 0:128])
        trB = np.trace(g[:, 128:256])
        trC = np.trace(g[:, 256:384])
        trD = np.trace(g[:, 384:512])
        # sum m*y^2: ACT/ttr cols for 'a' and tail tiles, psD for 'p' tiles
        myq = trD + sum(o[:, t].sum() for t in range(T)
                        if t >= n_pe_tiles or MYSQ[t] == 'a')
        cross = trA + sum(o[:, T + t].sum() for t in range(n_pe_tiles, T))
        mxq = trB + sum(o[:, 2 * T + t].sum() for t in range(n_pe_tiles, T))
        bg = ALPHA * trC + sum(o[:, 3 * T + t].sum()
                               for t in range(n_pe_tiles, T))
        total += myq - 2.0 * cross + (1.0 - ALPHA) * mxq + bg
    return np.float32(total)


# revision 11
# speedup vs baseline: 1.2195x; 1.0606x over previous
"""Masked-loss kernel for nn_MLoss_9715216024200 on 8 Trainium2 NeuronCores.

loss = sum(where(y[...,0]>0.5, (y-x)^2 - a*x^2, 0)) + a*sum(x[...,0]^2)
with x,y f32 (256, 10647, 5); output is a f32 scalar.

Sharding: flatten to cells, pad with 256 zero cells (neutral: y0=0 ->
mask 0, x=0 -> no bg term), split across (8 cores, 128 partitions,
2662 cells), and ship the shards as bf16 in FEATURE-PLANE layout: each
core's x (and y) is [128, 5*2662] with the 5 features stored as packed
per-feature planes.  bf16 halves the HBM stream to ~19us at the 360GB/s
DMA roofline (the loss tolerates it: rel err ~1e-4 << 2e-2), and the
plane layout makes the mask a PACKED [128, w] tensor -- no per-feature
replication, so the mask costs 0.28ns/cell (DVE tensor_scalar 4x mode)
instead of 1.4ns/elem on Pool.

Per-core math uses mask idempotence (m in {0,1} => m^2 = m):

  sum(m*(d^2 - a*x^2)) = sum(m*y^2) - 2*sum((m*x) o y) + (1-a)*sum((m*x)^2)

The big sums are Gram-matrix diagonals on the otherwise idle PE: per
128-cell block and feature plane, matmul(psA += mx_f^T y_f),
matmul(psB += mx_f^T x_f), matmul(psC += x0^T x0), and for 'p'-mode
tiles matmul(psD += m^T y2_f) (53ns per matmul; the host takes trace()
of the four staged 128x128 Grams).  PE matmuls are emitted PE_LAG tiles
behind the stream so PE ramps to full clock once and stays there.
sum(m*y^2) is per-tile routed by MYSQ: 'a' = my=m*y (part of one
10-plane masked multiply) + ACT Square-accum; 'p' = y2=y*y (DVE/ACT/
Pool by knob) + PE D-Gram.  The tail tile skips PE (DVE ttr cross +
my^2, ACT mx^2 + bg) so the Gram psums close at tile N-2 and their
export overlaps the drain.  Host combines everything in f64.

Per-elem rates: DVE tt/ts packed bf16 0.52/0.28, DVE ttr 1.05, ACT
square 0.83 (+0.6us/instr), Pool tt 1.98, PE 0.41 per Gram term.
Engines land at ~10-15us each, under the ~19.3us DMA stream.
"""
import sys

for _p in ('/opt/trn_rl_repo',):
    if _p in sys.path:
        sys.path.remove(_p)
    sys.path.insert(0, _p)

import os as _os
import numpy as np

B, C, F = 256, 10647, 5
THRESH = 0.5
ALPHA = 0.1
N_CORES = 8
P = 128
CELLS = B * C                      # 2,725,632
CPP = 2662                         # cells/partition; 8*128*2662 = 2,725,888
PAD_CELLS = N_CORES * P * CPP - CELLS   # 256
FD = CPP * F                       # 13310 elems per partition per core

_ts = _os.environ.get('TILE_SIZES', '')
TILE_SIZES = ([int(v) for v in _ts.split(',')] if _ts
              else [256] * 10 + [102])
assert sum(TILE_SIZES) == CPP
N_TILES = len(TILE_SIZES)
# per-pe-tile my^2 route: 'a' = my + ACT square-accum, 'p' = y2 + PE D-Gram
_mm = _os.environ.get('MYSQ', 'p,a,p,a,p,a,p,a,p,a,v')
MYSQ = _mm.split(',')
assert len(MYSQ) == N_TILES
# engine for y2 on 'p' tiles: tiles listed in Y2_ACT use ACT, Y2_POOL use
# Pool, the rest DVE
_ya = _os.environ.get('Y2_ACT', '8')
Y2_ACT = set(int(v) for v in _ya.split(',') if v != '')
_yp = _os.environ.get('Y2_POOL', '0,2,4')
Y2_POOL = set(int(v) for v in _yp.split(',') if v != '')
# 'a' tiles whose my-planes run on Pool (separate from the mx multiply)
_mg = _os.environ.get('MY_POOL', '1')
MY_POOL = set(int(v) for v in _mg.split(',') if v != '')
# how many trailing tiles skip PE
LAST_VEC = int(_os.environ.get('LAST_VEC', '1'))
# flush tile t's PE matmuls after tile t+PE_LAG's products are emitted
PE_LAG = int(_os.environ.get('PE_LAG', '2'))
BUFS = [int(v) for v in _os.environ.get('BUFS', '6,6,5,3').split(',')]

_compiled = None


def _build():
    from contextlib import ExitStack
    import concourse.tile as tile
    from concourse import bacc, mybir

    sqa = float(np.sqrt(ALPHA))

    nc = bacc.Bacc("TRN2", target_bir_lowering=False, debug=False,
                   enable_asserts=True, num_devices=N_CORES)
    bf16 = mybir.dt.bfloat16
    f32 = mybir.dt.float32
    x_d = nc.dram_tensor("x", [P, FD], bf16, kind="ExternalInput").ap()
    y_d = nc.dram_tensor("y", [P, FD], bf16, kind="ExternalInput").ap()
    o_d = nc.dram_tensor("o", [P, 4 * N_TILES], f32, kind="ExternalOutput").ap()
    g_d = nc.dram_tensor("g", [P, 512], f32, kind="ExternalOutput").ap()

    Sq = mybir.ActivationFunctionType.Square
    Alu = mybir.AluOpType

    n_pe = N_TILES - LAST_VEC
    last_p = max((t for t in range(n_pe) if MYSQ[t] == 'p'), default=-1)
    # psum col ranges: A=cross(mx,y)  B=(mx)^2  C=bg x0^2  D=m o y^2
    first_pe = [True, True, True, True]

    x3 = x_d.rearrange("p (f c) -> p f c", f=F)
    y3 = y_d.rearrange("p (f c) -> p f c", f=F)

    with tile.TileContext(nc) as tc, ExitStack() as ctx:
        xyp = ctx.enter_context(tc.tile_pool(name="xy", bufs=BUFS[0]))
        mp_ = ctx.enter_context(tc.tile_pool(name="m", bufs=BUFS[1]))
        wp = ctx.enter_context(tc.tile_pool(name="w", bufs=BUFS[2]))
        sp = ctx.enter_context(tc.tile_pool(name="s", bufs=BUFS[3]))
        ap_ = ctx.enter_context(tc.tile_pool(name="acc", bufs=1))
        pp = ctx.enter_context(tc.psum_pool(name="ps", bufs=1))

        acc = ap_.tile([P, 4 * N_TILES], f32)
        gst = ap_.tile([P, 512], f32)
        ps = pp.tile([P, 512], f32)

        def emit_pe(t, w, xy, m, mxv, y2v):
            # per 128-cell block: A/B per plane; C plane-0; D (if y2v) per
            # plane against the shared mask stationary
            last_ab = (t == n_pe - 1)
            nb = (w + 127) // 128
            for j in range(nb):
                lo = j * 128
                wb = min(128, w - lo)
                lab = last_ab and (j == nb - 1)
                for f in range(F):
                    mxf = mxv[:, f * w + lo: f * w + lo + wb]
                    yf = xy[:, (F + f) * w + lo: (F + f) * w + lo + wb]
                    xf = xy[:, f * w + lo: f * w + lo + wb]
                    nc.tensor.matmul(ps[0:wb, 0:wb], mxf, yf,
                                     start=first_pe[0],
                                     stop=lab and f == F - 1,
                                     skip_group_check=True)
                    first_pe[0] = False
                    nc.tensor.matmul(ps[0:wb, 128:128 + wb], mxf, xf,
                                     start=first_pe[1],
                                     stop=lab and f == F - 1,
                                     skip_group_check=True)
                    first_pe[1] = False
                    if y2v is not None:
                        y2f = y2v[:, f * w + lo: f * w + lo + wb]
                        nc.tensor.matmul(ps[0:wb, 384:384 + wb],
                                         m[:, lo:lo + wb], y2f,
                                         start=first_pe[3],
                                         stop=(t == last_p and j == nb - 1
                                               and f == F - 1),
                                         skip_group_check=True)
                        first_pe[3] = False
                x0b = xy[:, lo:lo + wb]
                nc.tensor.matmul(ps[0:wb, 256:256 + wb], x0b, x0b,
                                 start=first_pe[2], stop=lab,
                                 skip_group_check=True)
                first_pe[2] = False
            if last_ab:
                # stage Grams to SBUF + export (overlaps the tail)
                nc.vector.tensor_copy(gst[:], ps[:])
                nc.sync.dma_start(g_d, gst[:])

        pe_pending = []

        off = 0
        for t, w in enumerate(TILE_SIZES):
            xy = xyp.tile([P, 2 * F * w], bf16, tag="xy")
            sl = slice(off, off + w)
            off += w
            # y first (mask depends on it), then x; plane-strided DMAs
            nc.sync.dma_start(
                xy[:, F * w:2 * F * w].rearrange("p (f c) -> p f c", f=F),
                y3[:, :, sl])
            nc.sync.dma_start(
                xy[:, 0:F * w].rearrange("p (f c) -> p f c", f=F),
                x3[:, :, sl])
            xpl = xy[:, 0:F * w]          # x planes
            ypl = xy[:, F * w:2 * F * w]  # y planes

            # packed per-cell mask (4x DVE tensor_scalar)
            m = mp_.tile([P, w], bf16, tag="m")
            nc.vector.tensor_scalar(m[:], xy[:, F * w:F * w + w],
                                    THRESH, None, op0=Alu.is_gt)

            mode = MYSQ[t] if t < n_pe else 'v'
            y2v = None
            if mode == 'p':
                mxt = wp.tile([P, F * w], bf16, tag="mx")
                nc.vector.tensor_tensor(
                    mxt[:].rearrange("p (f c) -> p f c", f=F),
                    xpl.rearrange("p (f c) -> p f c", f=F),
                    m[:].unsqueeze(1).broadcast_to((P, F, w)), op=Alu.mult)
                mxv = mxt[:]
                y2t = wp.tile([P, F * w], bf16, tag="y2")
                y2_eng = (nc.scalar if t in Y2_ACT
                          else nc.gpsimd if t in Y2_POOL else nc.vector)
                if t in Y2_ACT:
                    nc.scalar.activation(y2t[:], ypl, Sq)
                else:
                    y2_eng.tensor_tensor(y2t[:], ypl, ypl, op=Alu.mult)
                y2v = y2t[:]
            else:
                # masked multiply of all 10 planes (or x/y split DVE/Pool)
                mxy = wp.tile([P, 2 * F * w], bf16, tag="mx")
                if t in MY_POOL and mode == 'a':
                    nc.vector.tensor_tensor(
                        mxy[:, 0:F * w].rearrange("p (f c) -> p f c", f=F),
                        xpl.rearrange("p (f c) -> p f c", f=F),
                        m[:].unsqueeze(1).broadcast_to((P, F, w)),
                        op=Alu.mult)
                    nc.gpsimd.tensor_tensor(
                        mxy[:, F * w:].rearrange("p (f c) -> p f c", f=F),
                        ypl.rearrange("p (f c) -> p f c", f=F),
                        m[:].unsqueeze(1).broadcast_to((P, F, w)),
                        op=Alu.mult)
                else:
                    nc.vector.tensor_tensor(
                        mxy[:].rearrange("p (k c) -> p k c", k=2 * F),
                        xy[:].rearrange("p (k c) -> p k c", k=2 * F),
                        m[:].unsqueeze(1).broadcast_to((P, 2 * F, w)),
                        op=Alu.mult)
                mxv = mxy[:, 0:F * w]
                myv = mxy[:, F * w:2 * F * w]
                if mode == 'a':
                    sq = sp.tile([P, F * w], bf16, tag="sq")
                    nc.scalar.activation(sq[:], myv, Sq,
                                         accum_out=acc[:, t:t + 1])

            if t < n_pe:
                pe_pending.append((t, w, xy, m, mxv, y2v))
                while pe_pending and (pe_pending[0][0] + PE_LAG <= t
                                      or t == n_pe - 1):
                    emit_pe(*pe_pending.pop(0))
            else:
                # tail tile off PE: cross + my^2 on DVE ttr, mx^2 + bg on ACT
                cw = sp.tile([P, F * w], bf16, tag="cw")
                nc.vector.tensor_tensor_reduce(
                    cw[:], mxv, ypl, 1.0, 0.0, op0=Alu.mult, op1=Alu.add,
                    accum_out=acc[:, N_TILES + t:N_TILES + t + 1])
                cw2 = sp.tile([P, F * w], bf16, tag="cw2")
                nc.vector.tensor_tensor_reduce(
                    cw2[:], myv, myv, 1.0, 0.0, op0=Alu.mult, op1=Alu.add,
                    accum_out=acc[:, t:t + 1])
                sq2 = sp.tile([P, F * w], bf16, tag="sq2")
                nc.scalar.activation(sq2[:], mxv, Sq,
                                     accum_out=acc[:, 2 * N_TILES + t:
                                                   2 * N_TILES + t + 1])
                sq3 = sp.tile([P, w], bf16, tag="sq3")
                nc.scalar.activation(sq3[:], xy[:, 0:w], Sq, scale=sqa,
                                     accum_out=acc[:, 3 * N_TILES + t:
                                                   3 * N_TILES + t + 1])

        nc.sync.dma_start(o_d, acc[:])

    nc.compile()
    return nc


def _shard(a: np.ndarray) -> list[np.ndarray]:
    import ml_dtypes
    flat = a.reshape(-1)
    pad = np.zeros(PAD_CELLS * F, dtype=a.dtype)
    flat = np.concatenate([flat, pad]).astype(ml_dtypes.bfloat16)
    # (cores, P, cells, F) -> feature-plane layout (cores, P, F, cells)
    pc = flat.reshape(N_CORES, P, CPP, F).transpose(0, 1, 3, 2)
    pc = pc.reshape(N_CORES, P, FD)
    return [np.ascontiguousarray(pc[i]) for i in range(N_CORES)]


def kernel(x: np.ndarray, y: np.ndarray) -> np.ndarray:
    global _compiled
    if _compiled is None:
        _compiled = _build()
    nc = _compiled

    from concourse.bass_utils import run_bass_kernel_spmd

    xs = _shard(np.asarray(x, dtype=np.float32))
    ys = _shard(np.asarray(y, dtype=np.float32))
    in_maps = [{"x": xs[i], "y": ys[i]} for i in range(N_CORES)]
    res = run_bass_kernel_spmd(nc, in_maps, core_ids=list(range(N_CORES)))

    T = N_TILES
    n_pe = T - LAST_VEC
    total = np.float64(0.0)
    for r in res.results:
        o = r["o"].astype(np.float64)
        g = r["g"].astype(np.float64)
        trA = np.trace(g[:, 0:128])
        trB = np.trace(g[:, 128:256])
        trC = np.trace(g[:, 256:384])
        trD = np.trace(g[:, 384:512])
        myq = trD + sum(o[:, t].sum() for t in range(T)
                        if t >= n_pe or MYSQ[t] == 'a')
        cross = trA + sum(o[:, T + t].sum() for t in range(n_pe, T))
        mxq = trB + sum(o[:, 2 * T + t].sum() for t in range(n_pe, T))
        bg = ALPHA * trC + sum(o[:, 3 * T + t].sum() for t in range(n_pe, T))
        total += myq - 2.0 * cross + (1.0 - ALPHA) * mxq + bg
    return np.float32(total)
